# revision 67
# baseline (speedup 1.0000x reference)
"""LoRA basis-bank kernel for 8 TRN2 NeuronCores.

Math (per batch b):
    A_mixed  = sum_k alpha[b,k] * A_bank[k]        # [R, DIN]
    B_mixedT = sum_k alpha[b,k] * B_bank[k].T      # [R, DOUT]
    z        = h[b] @ A_mixed.T                    # [S, R]
    delta[b] = z @ B_mixedT                        # [S, DOUT]

Sharding: data-parallel over batch, 1 batch per core; banks replicated.

Memory traffic per core: 8MB hT + ~2.3MB banks in, 8MB delta out
(bf16).  All DMAs are ~1MB with contiguous 8KB rows: hT is uploaded
pre-tiled so each 512-row s-chunk is two [128, 4096] loads (sync ring);
delta is stored as two [128, 4096] DMAs per s-chunk (scalar ring) and
untiled on host.

PE work is minimized because the PE clock duty-cycles between 1.2 and
2.4 GHz under 8-core load:
  - mm1: zT[16, 512] accumulated over 16 DIN chunks (N=512 streams,
    16-col weight loads that pull ahead).
  - mm2 is 4x row-packed: zT is partition-scattered on the PE itself
    (4 accumulating matmuls against constant selector matrices E_t)
    into zt4[128, 128] with s-tile t at partitions 32t..32t+15, and
    B_mixedT is computed 4x-replicated (bmix4) via a host-built
    placement matrix, so four [16]-contraction matmuls run concurrently
    on distinct PE row strips (tile_position=(32t, 0)).

Schedule: software-pipelined — mm1 of chunk ch+1 is interleaved between
mm2 waves of chunk ch so the PSUM->SBUF copy drain (split per-wave
between DVE and ACT so both engines run concurrently) hides under mm1.
A ~4us dummy-matmul warm-up at t=0 and warm-keeper matmuls in the
copy-gated last chunk keep the HAM clock gate at 2.4GHz.
"""

import ml_dtypes
import numpy as np

import concourse.bacc as bacc
import concourse.bass as bass
import concourse.mybir as mybir
import concourse.tile as tile
from concourse.bass_utils import run_bass_kernel_spmd

B, S, K, R, DIN, DOUT = 8, 2048, 16, 16, 2048, 2048
KR = K * R  # 256
F32 = mybir.dt.float32
BF16 = mybir.dt.bfloat16

NCH = DIN // 128   # 16 DIN chunks
SC = 512           # s-chunk rows
NSC = S // SC      # 4 s-chunks

_cache = {}


def _build_nc():
    nc = bacc.Bacc("TRN2", target_bir_lowering=False)

    # hT pre-tiled: row ch*128+p, col c*512+j  <-  h[ch*512+j, c*128+p]
    ht_d = nc.dram_tensor("hbT", [NSC * 128, NCH * SC], BF16,
                          kind="ExternalInput")
    # [a_half0 | a_half1 | mixrep_half0 | mixrep_half1 | E_scatter]
    am_d = nc.dram_tensor("am", [128, 2 * DIN + 2 * 128 + 512], BF16,
                          kind="ExternalInput")
    # B^T halves packed: [b0 | b1]
    bb_d = nc.dram_tensor("bb", [128, DOUT * 2], BF16, kind="ExternalInput")
    # delta tiled: row ch*128+p, col half*4096 + t*1024 + q*512 + j
    #   -> delta[ch*512 + t*128 + p, (half*2+q)*512 + j]
    out_d = nc.dram_tensor("delta", [NSC * 128, 4 * DOUT], BF16,
                           kind="ExternalOutput")
    # tiny sink so the HAM warm-up matmuls aren't dead code
    warm_d = nc.dram_tensor("warm", [R, R], BF16, kind="ExternalOutput")

    MR0 = 2 * DIN            # mixrep half0 col offset
    MR1 = 2 * DIN + 128      # mixrep half1 col offset
    E0 = 2 * DIN + 256       # E_scatter col offset: E[r, t*128+m]=1 iff m==32t+r

    with tile.TileContext(nc) as tc:
        with (
            tc.tile_pool(name="const", bufs=1) as constp,
            tc.tile_pool(name="hT", bufs=8) as hTp,
            tc.tile_pool(name="zz", bufs=2) as zp,
            tc.tile_pool(name="z4", bufs=2) as z4p,
            tc.tile_pool(name="dout", bufs=3) as dp,
            tc.tile_pool(name="aux", bufs=2, space="PSUM") as auxp,
            tc.tile_pool(name="psd", bufs=6, space="PSUM") as psdp,
        ):
            def load_hT_half(ch, half):
                hT = hTp.tile([128, NCH * SC // 2], BF16, tag="hT")
                nc.sync.dma_start(
                    hT[:], ht_d[ch * 128:(ch + 1) * 128,
                                half * 4096:(half + 1) * 4096])
                return hT

            # ---- HAM warm-up: ~4us of dummy matmuls with no DMA deps so
            # the PE clock is at 2.4GHz by the time real work arrives ----
            wtile = constp.tile([128, 512], BF16, tag="wrm")
            nc.vector.memset(wtile[:], 0.5)
            wps = None
            for i in range(12):
                wps = auxp.tile([128, 512], F32, tag="aux")
                nc.tensor.matmul(wps[:R, :], wtile[:, :R], wtile[:])
            wsb = constp.tile([R, R], BF16, tag="wsb")
            nc.vector.tensor_copy(wsb[:], wps[:R, :R])
            nc.scalar.dma_start(warm_d[:, :], wsb[:])

            # ---- sync-ring issue order: am, h0a, h0b, h1a, h1b, bb, .. ----
            am_sb = constp.tile([128, 2 * DIN + 2 * 128 + 512], BF16, tag="am")
            nc.sync.dma_start(am_sb[:], am_d[:, :])
            hTs = {}
            hTs[(0, 0)] = load_hT_half(0, 0)
            hTs[(0, 1)] = load_hT_half(0, 1)
            bb_sb = constp.tile([128, 2 * DOUT], BF16, tag="bb")
            nc.sync.dma_start(bb_sb[:], bb_d[:, :])
            for ch in range(1, NSC):
                hTs[(ch, 0)] = load_hT_half(ch, 0)
                hTs[(ch, 1)] = load_hT_half(ch, 1)

            m0 = am_sb[:, MR0:MR0 + R]      # plain mix half0 (t=0 block)
            m1 = am_sb[:, MR1:MR1 + R]

            # ---- A_mixT chunks: [128, 16] = A_half_chunk.T @ M_half ----
            amixT = constp.tile([128, NCH * R], BF16, tag="amixT")
            for c in range(NCH):
                pat = psdp.tile([128, R], F32, tag="dps")
                nc.tensor.matmul(pat[:], am_sb[:, c * 128:(c + 1) * 128],
                                 m0, start=True, stop=False)
                nc.tensor.matmul(pat[:], am_sb[:, DIN + c * 128:
                                               DIN + (c + 1) * 128],
                                 m1, start=False, stop=True)
                nc.vector.tensor_copy(amixT[:, c * R:(c + 1) * R], pat[:])

            # ---- main loop over 512-row s-chunks, software-pipelined:
            # mm1 of chunk ch+1 is interleaved between mm2 waves of chunk
            # ch so the PE never stalls on the PSUM->SBUF copy drain ----
            def mm1_quarter(ch, zt_ps, w):
                for c in range(4 * w, 4 * w + 4):
                    hT = hTs[(ch, c // 8)]
                    nc.tensor.matmul(zt_ps[:], amixT[:, c * R:(c + 1) * R],
                                     hT[:, (c % 8) * 512:(c % 8 + 1) * 512],
                                     start=(c == 0), stop=(c == NCH - 1),
                                     skip_group_check=True)

            def zt_path(ch, zt_ps):
                zt = zp.tile([R, SC], BF16, tag="z")
                nc.vector.tensor_copy(zt[:], zt_ps[:])
                # partition-scatter zT -> zt4 on the PE: s-tile t lands at
                # partitions 32t..32t+15 via constant selector matmuls
                zt4_ps = auxp.tile([128, 128], F32, tag="aux")
                for t in range(4):
                    nc.tensor.matmul(
                        zt4_ps[:],
                        am_sb[0:R, E0 + t * 128:E0 + (t + 1) * 128],
                        zt[:, t * 128:(t + 1) * 128],
                        start=(t == 0), stop=(t == 3),
                        skip_group_check=True)
                zt4 = z4p.tile([128, 128], BF16, tag="z4")
                nc.scalar.copy(zt4[:], zt4_ps[:])
                return zt4

            zt_ps0 = auxp.tile([R, SC], F32, tag="aux")
            for w in range(4):
                mm1_quarter(0, zt_ps0, w)
            zt4 = zt_path(0, zt_ps0)

            # ---- bmix4 [128, DOUT]: row 32t+r = B_mixedT[r, :] ----
            # (emitted after mm1(0) so the PE isn't queued behind the bb load)
            bmix4 = constp.tile([128, DOUT], BF16, tag="bmix4")
            for oc in range(DOUT // 512):
                sl = slice(oc * 512, (oc + 1) * 512)
                pb4 = psdp.tile([128, 512], F32, tag="dps")
                nc.tensor.matmul(pb4[:], am_sb[:, MR0:MR0 + 128],
                                 bb_sb[:, sl], start=True, stop=False)
                nc.tensor.matmul(pb4[:], am_sb[:, MR1:MR1 + 128],
                                 bb_sb[:, DOUT + oc * 512:
                                       DOUT + (oc + 1) * 512],
                                 start=False, stop=True)
                nc.vector.tensor_copy(bmix4[:, sl], pb4[:])

            for ch in range(NSC):
                dsb = []
                for h in range(2):
                    dtile = dp.tile([128, 2 * DOUT], BF16, tag=f"d{h}")
                    dsb.append(dtile)
                if ch + 1 < NSC:
                    zt_ps = auxp.tile([R, SC], F32, tag="aux")
                last = ch + 1 == NSC
                for oc in range(DOUT // 512):
                    half, q = oc // 2, oc % 2
                    wave = []
                    for t in range(4):
                        dps = psdp.tile([128, 512], F32, tag="dps")
                        nc.tensor.matmul(
                            dps[:], zt4[32 * t:32 * t + R, 0:128],
                            bmix4[32 * t:32 * t + R,
                                  oc * 512:(oc + 1) * 512],
                            tile_position=(32 * t, 0))
                        wave.append(dps)
                    if not last:
                        mm1_quarter(ch + 1, zt_ps, oc)
                        if oc == 3:
                            # zt copy first in the DVE queue so the next
                            # chunk's scatter isn't stuck behind wave copies
                            zt4_next = zt_path(ch + 1, zt_ps)
                    elif oc < 3:
                        # keep the PE dense (and HAM warm) through the
                        # copy-gated last chunk
                        for i in range(3):
                            wps = auxp.tile([128, 512], F32, tag="aux")
                            nc.tensor.matmul(wps[:R, :], wtile[:, :R],
                                             wtile[:])
                        if oc == 2:
                            wsb2 = constp.tile([R, R], BF16, tag="wsb")
                            nc.vector.tensor_copy(wsb2[:], wps[:R, :R])
                            nc.scalar.dma_start(warm_d[:, :], wsb2[:])
                    for t in range(4):
                        dps = wave[t]
                        osl = slice(t * 1024 + q * 512,
                                    t * 1024 + (q + 1) * 512)
                        # split within each wave so DVE and ACT drain
                        # concurrently instead of in phases
                        if t < 2:
                            nc.vector.tensor_copy(dsb[half][:, osl], dps[:])
                        else:
                            nc.scalar.copy(dsb[half][:, osl], dps[:])
                    if last:
                        # store each wave's strided quarter immediately,
                        # on the long-idle sync ring; final wave split in
                        # two so the DVE half ships without waiting on ACT
                        dst3 = (out_d[ch * 128:(ch + 1) * 128,
                                      half * 4096:(half + 1) * 4096]
                                .rearrange("p (t r2) -> p t r2", t=4)
                                [:, :, q * 512:(q + 1) * 512])
                        src3 = (dsb[half][:]
                                .rearrange("p (t r2) -> p t r2", t=4)
                                [:, :, q * 512:(q + 1) * 512])
                        if oc == 3:
                            nc.sync.dma_start(dst3[:, 0:2, :],
                                              src3[:, 0:2, :])
                            nc.sync.dma_start(dst3[:, 2:4, :],
                                              src3[:, 2:4, :])
                        else:
                            nc.sync.dma_start(dst3, src3)
                    elif oc == 1:
                        nc.scalar.dma_start(
                            out_d[ch * 128:(ch + 1) * 128, 0:4096],
                            dsb[0][:])
                if not last:
                    zt4 = zt4_next
                    nc.scalar.dma_start(
                        out_d[ch * 128:(ch + 1) * 128, 4096:8192],
                        dsb[1][:])

    nc.compile()
    return nc


def _in_maps(h, alpha, A_bank, B_bank):
    a_flat = np.ascontiguousarray(
        A_bank.reshape(KR, DIN)).astype(ml_dtypes.bfloat16)
    bt_flat = np.ascontiguousarray(
        B_bank.transpose(0, 2, 1).reshape(KR, DOUT)).astype(ml_dtypes.bfloat16)
    bb = np.ascontiguousarray(
        np.concatenate([bt_flat[:128], bt_flat[128:]], axis=1))
    eye = np.eye(R, dtype=np.float32)
    maps = []
    for b in range(B):
        mix = np.kron(alpha[b].astype(np.float32).reshape(K, 1), eye)
        mrep = np.zeros((KR, 128), np.float32)
        for t in range(4):
            mrep[:, 32 * t:32 * t + R] = mix
        mrep = mrep.astype(ml_dtypes.bfloat16)
        E = np.zeros((128, 512), np.float32)
        for t in range(4):
            for r in range(R):
                E[r, t * 128 + 32 * t + r] = 1.0
        E = E.astype(ml_dtypes.bfloat16)
        am = np.ascontiguousarray(np.concatenate(
            [a_flat[:128], a_flat[128:], mrep[:128], mrep[128:], E], axis=1))
        hT = np.ascontiguousarray(
            np.asarray(h[b]).reshape(NSC, SC, NCH, 128)
            .transpose(0, 3, 2, 1).reshape(NSC * 128, NCH * SC)
        ).astype(ml_dtypes.bfloat16)
        maps.append({"hbT": hT, "am": am, "bb": bb})
    return maps


def _run(inputs, trace=False):
    if "nc" not in _cache:
        _cache["nc"] = _build_nc()
    nc = _cache["nc"]
    maps = _in_maps(inputs["h"], inputs["alpha"], inputs["A_bank"],
                    inputs["B_bank"])
    res = run_bass_kernel_spmd(nc, maps, core_ids=list(range(B)), trace=trace)
    out = np.stack(
        [res.results[b]["delta"]
         .reshape(NSC, 128, 2, 4, 2, 512).transpose(0, 3, 1, 2, 4, 5)
         .reshape(S, DOUT) for b in range(B)], axis=0)
    return out.astype(np.float32), res


def kernel(**inputs):
    out, _ = _run(inputs, trace=False)
    return out


# revision 69
# speedup vs baseline: 1.0775x; 1.0775x over previous
"""LoRA basis-bank kernel for 8 TRN2 NeuronCores.

Math (per batch b):
    A_mixed  = sum_k alpha[b,k] * A_bank[k]        # [R, DIN]
    B_mixedT = sum_k alpha[b,k] * B_bank[k].T      # [R, DOUT]
    z        = h[b] @ A_mixed.T                    # [S, R]
    delta[b] = z @ B_mixedT                        # [S, DOUT]

Sharding: data-parallel over batch, 1 batch per core; banks replicated.

Memory traffic per core: 8MB hT + ~2.3MB banks in, 8MB delta out
(bf16).  All DMAs are ~1MB with contiguous 8KB rows: hT is uploaded
pre-tiled so each 512-row s-chunk is two [128, 4096] loads (sync ring);
delta is stored as two [128, 4096] DMAs per s-chunk (scalar ring) and
untiled on host.

PE work is minimized because the PE clock duty-cycles between 1.2 and
2.4 GHz under 8-core load:
  - mm1: zT[16, 512] accumulated over 16 DIN chunks (N=512 streams,
    16-col weight loads that pull ahead).
  - mm2 is 4x row-packed: zT is partition-scattered on the PE itself
    (4 accumulating matmuls against constant selector matrices E_t)
    into zt4[128, 128] with s-tile t at partitions 32t..32t+15, and
    B_mixedT is computed 4x-replicated (bmix4) via a host-built
    placement matrix, so four [16]-contraction matmuls run concurrently
    on distinct PE row strips (tile_position=(32t, 0)).

Schedule: software-pipelined — mm1 of chunk ch+1 is interleaved between
mm2 waves of chunk ch so the PSUM->SBUF copy drain (split per-wave
between DVE and ACT so both engines run concurrently) hides under mm1.
A ~4us dummy-matmul warm-up at t=0 and warm-keeper matmuls in the
copy-gated last chunk keep the HAM clock gate at 2.4GHz.
"""

import ml_dtypes
import numpy as np

import concourse.bacc as bacc
import concourse.bass as bass
import concourse.mybir as mybir
import concourse.tile as tile
from concourse.bass_utils import run_bass_kernel_spmd

B, S, K, R, DIN, DOUT = 8, 2048, 16, 16, 2048, 2048
KR = K * R  # 256
F32 = mybir.dt.float32
BF16 = mybir.dt.bfloat16

NCH = DIN // 128   # 16 DIN chunks
SC = 512           # s-chunk rows
NSC = S // SC      # 4 s-chunks

_cache = {}


def _build_nc():
    nc = bacc.Bacc("TRN2", target_bir_lowering=False)

    # hT pre-tiled: row ch*128+p, col c*512+j  <-  h[ch*512+j, c*128+p]
    ht_d = nc.dram_tensor("hbT", [NSC * 128, NCH * SC], BF16,
                          kind="ExternalInput")
    # [a_half0 | a_half1 | mixrep_half0 | mixrep_half1 | E_scatter]
    am_d = nc.dram_tensor("am", [128, 2 * DIN + 2 * 128 + 512], BF16,
                          kind="ExternalInput")
    # B^T halves packed: [b0 | b1]
    bb_d = nc.dram_tensor("bb", [128, DOUT * 2], BF16, kind="ExternalInput")
    # delta tiled: row ch*128+p, col half*4096 + t*1024 + q*512 + j
    #   -> delta[ch*512 + t*128 + p, (half*2+q)*512 + j]
    out_d = nc.dram_tensor("delta", [NSC * 128, 4 * DOUT], BF16,
                           kind="ExternalOutput")
    # tiny sink so the HAM warm-up matmuls aren't dead code
    warm_d = nc.dram_tensor("warm", [R, R], BF16, kind="ExternalOutput")

    MR0 = 2 * DIN            # mixrep half0 col offset
    MR1 = 2 * DIN + 128      # mixrep half1 col offset
    E0 = 2 * DIN + 256       # E_scatter col offset: E[r, t*128+m]=1 iff m==32t+r

    with tile.TileContext(nc) as tc:
        with (
            tc.tile_pool(name="const", bufs=1) as constp,
            tc.tile_pool(name="hT", bufs=8) as hTp,
            tc.tile_pool(name="zz", bufs=2) as zp,
            tc.tile_pool(name="z4", bufs=2) as z4p,
            tc.tile_pool(name="dout", bufs=3) as dp,
            tc.tile_pool(name="aux", bufs=1, space="PSUM") as auxp,
            tc.tile_pool(name="psd", bufs=7, space="PSUM") as psdp,
        ):
            def load_hT_half(ch, half):
                hT = hTp.tile([128, NCH * SC // 2], BF16, tag="hT")
                nc.sync.dma_start(
                    hT[:], ht_d[ch * 128:(ch + 1) * 128,
                                half * 4096:(half + 1) * 4096])
                return hT

            # ---- HAM warm-up: ~4us of dummy matmuls with no DMA deps so
            # the PE clock is at 2.4GHz by the time real work arrives ----
            wtile = constp.tile([128, 512], BF16, tag="wrm")
            nc.vector.memset(wtile[:], 0.5)
            wps = None
            for i in range(10):
                wps = auxp.tile([128, 512], F32, tag="aux")
                nc.tensor.matmul(wps[:R, :], wtile[:, :R], wtile[:])
            wsb = constp.tile([R, R], BF16, tag="wsb")
            nc.vector.tensor_copy(wsb[:], wps[:R, :R])
            nc.scalar.dma_start(warm_d[:, :], wsb[:])

            # ---- sync-ring issue order: am, h0a, h0b, h1a, h1b, bb, .. ----
            am_sb = constp.tile([128, 2 * DIN + 2 * 128 + 512], BF16, tag="am")
            nc.sync.dma_start(am_sb[:], am_d[:, :])
            hTs = {}
            hTs[(0, 0)] = load_hT_half(0, 0)
            hTs[(0, 1)] = load_hT_half(0, 1)
            bb_sb = constp.tile([128, 2 * DOUT], BF16, tag="bb")
            nc.sync.dma_start(bb_sb[:], bb_d[:, :])
            for ch in range(1, NSC):
                hTs[(ch, 0)] = load_hT_half(ch, 0)
                hTs[(ch, 1)] = load_hT_half(ch, 1)

            m0 = am_sb[:, MR0:MR0 + R]      # plain mix half0 (t=0 block)
            m1 = am_sb[:, MR1:MR1 + R]

            # ---- A_mixT chunks: [128, 16] = A_half_chunk.T @ M_half ----
            amixT = constp.tile([128, NCH * R], BF16, tag="amixT")
            for c in range(NCH):
                pat = psdp.tile([128, R], F32, tag="dps")
                nc.tensor.matmul(pat[:], am_sb[:, c * 128:(c + 1) * 128],
                                 m0, start=True, stop=False)
                nc.tensor.matmul(pat[:], am_sb[:, DIN + c * 128:
                                               DIN + (c + 1) * 128],
                                 m1, start=False, stop=True)
                nc.vector.tensor_copy(amixT[:, c * R:(c + 1) * R], pat[:])

            # ---- main loop over 512-row s-chunks, software-pipelined:
            # mm1 of chunk ch+1 is interleaved between mm2 waves of chunk
            # ch so the PE never stalls on the PSUM->SBUF copy drain ----
            def mm1_quarter(ch, zt_ps, w):
                for c in range(4 * w, 4 * w + 4):
                    hT = hTs[(ch, c // 8)]
                    nc.tensor.matmul(zt_ps[:], amixT[:, c * R:(c + 1) * R],
                                     hT[:, (c % 8) * 512:(c % 8 + 1) * 512],
                                     start=(c == 0), stop=(c == NCH - 1),
                                     skip_group_check=True)

            def zt_path(ch, zt_ps):
                zt = zp.tile([R, SC], BF16, tag="z")
                nc.vector.tensor_copy(zt[:], zt_ps[:])
                # partition-scatter zT -> zt4 on the PE: s-tile t lands at
                # partitions 32t..32t+15 via constant selector matmuls
                zt4_ps = auxp.tile([128, 128], F32, tag="aux")
                for t in range(4):
                    nc.tensor.matmul(
                        zt4_ps[:],
                        am_sb[0:R, E0 + t * 128:E0 + (t + 1) * 128],
                        zt[:, t * 128:(t + 1) * 128],
                        start=(t == 0), stop=(t == 3),
                        skip_group_check=True)
                zt4 = z4p.tile([128, 128], BF16, tag="z4")
                nc.scalar.copy(zt4[:], zt4_ps[:])
                return zt4

            zt_ps0 = auxp.tile([R, SC], F32, tag="aux")
            for w in range(4):
                mm1_quarter(0, zt_ps0, w)
            zt4 = zt_path(0, zt_ps0)

            # ---- bmix4 [128, DOUT]: row 32t+r = B_mixedT[r, :] ----
            # (emitted after mm1(0) so the PE isn't queued behind the bb load)
            bmix4 = constp.tile([128, DOUT], BF16, tag="bmix4")
            for oc in range(DOUT // 512):
                sl = slice(oc * 512, (oc + 1) * 512)
                pb4 = psdp.tile([128, 512], F32, tag="dps")
                nc.tensor.matmul(pb4[:], am_sb[:, MR0:MR0 + 128],
                                 bb_sb[:, sl], start=True, stop=False)
                nc.tensor.matmul(pb4[:], am_sb[:, MR1:MR1 + 128],
                                 bb_sb[:, DOUT + oc * 512:
                                       DOUT + (oc + 1) * 512],
                                 start=False, stop=True)
                nc.vector.tensor_copy(bmix4[:, sl], pb4[:])

            for ch in range(NSC):
                dsb = []
                for h in range(2):
                    dtile = dp.tile([128, 2 * DOUT], BF16, tag=f"d{h}")
                    dsb.append(dtile)
                if ch + 1 < NSC:
                    zt_ps = auxp.tile([R, SC], F32, tag="aux")
                last = ch + 1 == NSC
                for oc in range(DOUT // 512):
                    half, q = oc // 2, oc % 2
                    wave = []
                    for t in range(4):
                        dps = psdp.tile([128, 512], F32, tag="dps")
                        nc.tensor.matmul(
                            dps[:], zt4[32 * t:32 * t + R, 0:128],
                            bmix4[32 * t:32 * t + R,
                                  oc * 512:(oc + 1) * 512],
                            tile_position=(32 * t, 0))
                        wave.append(dps)
                    if not last:
                        mm1_quarter(ch + 1, zt_ps, oc)
                        if oc == 3:
                            # zt copy first in the DVE queue so the next
                            # chunk's scatter isn't stuck behind wave copies
                            zt4_next = zt_path(ch + 1, zt_ps)
                    elif oc < 3:
                        # keep the PE dense (and HAM warm) through the
                        # copy-gated last chunk
                        for i in range(3):
                            wps = auxp.tile([128, 512], F32, tag="aux")
                            nc.tensor.matmul(wps[:R, :], wtile[:, :R],
                                             wtile[:])
                        if oc == 2:
                            wsb2 = constp.tile([R, R], BF16, tag="wsb")
                            nc.vector.tensor_copy(wsb2[:], wps[:R, :R])
                            nc.scalar.dma_start(warm_d[:, :], wsb2[:])
                    for t in range(4):
                        dps = wave[t]
                        osl = slice(t * 1024 + q * 512,
                                    t * 1024 + (q + 1) * 512)
                        # split within each wave so DVE and ACT drain
                        # concurrently instead of in phases
                        if t < 2:
                            nc.vector.tensor_copy(dsb[half][:, osl], dps[:])
                        else:
                            nc.scalar.copy(dsb[half][:, osl], dps[:])
                    if last:
                        # store each wave's strided quarter immediately,
                        # on the long-idle sync ring; final wave split in
                        # two so the DVE half ships without waiting on ACT
                        dst3 = (out_d[ch * 128:(ch + 1) * 128,
                                      half * 4096:(half + 1) * 4096]
                                .rearrange("p (t r2) -> p t r2", t=4)
                                [:, :, q * 512:(q + 1) * 512])
                        src3 = (dsb[half][:]
                                .rearrange("p (t r2) -> p t r2", t=4)
                                [:, :, q * 512:(q + 1) * 512])
                        if oc == 3:
                            nc.sync.dma_start(dst3[:, 0:2, :],
                                              src3[:, 0:2, :])
                            nc.sync.dma_start(dst3[:, 2:4, :],
                                              src3[:, 2:4, :])
                        else:
                            nc.sync.dma_start(dst3, src3)
                    elif oc == 1:
                        nc.scalar.dma_start(
                            out_d[ch * 128:(ch + 1) * 128, 0:4096],
                            dsb[0][:])
                if not last:
                    zt4 = zt4_next
                    nc.scalar.dma_start(
                        out_d[ch * 128:(ch + 1) * 128, 4096:8192],
                        dsb[1][:])

    nc.compile()
    return nc


def _in_maps(h, alpha, A_bank, B_bank):
    a_flat = np.ascontiguousarray(
        A_bank.reshape(KR, DIN)).astype(ml_dtypes.bfloat16)
    bt_flat = np.ascontiguousarray(
        B_bank.transpose(0, 2, 1).reshape(KR, DOUT)).astype(ml_dtypes.bfloat16)
    bb = np.ascontiguousarray(
        np.concatenate([bt_flat[:128], bt_flat[128:]], axis=1))
    eye = np.eye(R, dtype=np.float32)
    maps = []
    for b in range(B):
        mix = np.kron(alpha[b].astype(np.float32).reshape(K, 1), eye)
        mrep = np.zeros((KR, 128), np.float32)
        for t in range(4):
            mrep[:, 32 * t:32 * t + R] = mix
        mrep = mrep.astype(ml_dtypes.bfloat16)
        E = np.zeros((128, 512), np.float32)
        for t in range(4):
            for r in range(R):
                E[r, t * 128 + 32 * t + r] = 1.0
        E = E.astype(ml_dtypes.bfloat16)
        am = np.ascontiguousarray(np.concatenate(
            [a_flat[:128], a_flat[128:], mrep[:128], mrep[128:], E], axis=1))
        hT = np.ascontiguousarray(
            np.asarray(h[b]).reshape(NSC, SC, NCH, 128)
            .transpose(0, 3, 2, 1).reshape(NSC * 128, NCH * SC)
        ).astype(ml_dtypes.bfloat16)
        maps.append({"hbT": hT, "am": am, "bb": bb})
    return maps


def _run(inputs, trace=False):
    if "nc" not in _cache:
        _cache["nc"] = _build_nc()
    nc = _cache["nc"]
    maps = _in_maps(inputs["h"], inputs["alpha"], inputs["A_bank"],
                    inputs["B_bank"])
    res = run_bass_kernel_spmd(nc, maps, core_ids=list(range(B)), trace=trace)
    out = np.stack(
        [res.results[b]["delta"]
         .reshape(NSC, 128, 2, 4, 2, 512).transpose(0, 3, 1, 2, 4, 5)
         .reshape(S, DOUT) for b in range(B)], axis=0)
    return out.astype(np.float32), res


def kernel(**inputs):
    out, _ = _run(inputs, trace=False)
    return out
